# revision 18
# baseline (speedup 1.0000x reference)
"""FlowNet correlation (kernel_size=1, max_displacement=4) on 8 Trainium2 cores.

Problem: input1, input2: [16, 256, 96, 96] fp32
         out[b, d, y, x] = (1/256) * sum_c in1[b,c,y,x] * in2pad[b,c,y+di,x+dj]
         d = (di+4)*9 + (dj+4), di,dj in [-4,4]  -> 81 output channels.

Sharding: data-parallel over batch, 2 samples per core, no collectives.

Per-core algorithm (cost-model-roofline driven: the single 360 GB/s
DMA-engines resource is the bottleneck, so the kernel minimizes DMA bytes
and instruction count and keeps the transfer queue saturated end-to-end):
  - in2 is DMA-cast fp32->bf16 into flat [128, 96*96] SBUF tiles (2
    contraction chunks); in1 likewise, staged through row-piece chunks and
    engine-copied to block-major (the matmul's stationary operand must be
    a contiguous [128, 96] slice -- walrus checkMatmultInputs rejects
    strided lhsT).  Loads arrive in row-thirds, interleaved across the 2
    batches, so compute starts early and trailing groups wait only on the
    last piece.
  - Per 8x12 pixel block: TensorE psum[m, n] = sum_c in1[c, m] * in2[c, n]
    with m over the 96 block pixels (stationary; stationary-dim waste is
    free, matmul cost is moving-dim cycles only) and n over the block's
    halo window CLAMPED to the image (<= 16x20 = 320 columns), read as a
    strided AP straight from the flat in2 tile.  2 accumulating bf16
    matmuls (C = 2 x 128).
  - ScalarE/VectorE copy psum -> a per-group SBUF staging tile (bf16).
  - One HWDGE DMA per group of 2x8 blocks writes the raw windows to DRAM
    (bf16).  No de-shear on device: the 81-of-window diagonal gather (a
    per-partition "sheared" pattern no compute engine can address and DMA
    only handles at <512B-descriptor half bandwidth) runs on the host,
    fully vectorized, with the exact *2^-8 scaling, zero-fill of
    out-of-image displacements, and the layout transpose.
"""

import numpy as np

import concourse.bass as bass
import concourse.mybir as mybir
import concourse.tile as tile
from concourse import bacc
from concourse import bass_utils
import bass_rust

MD = 4
B, C, H, W = 16, 256, 96, 96
NCORES = 8
BPC = B // NCORES          # batches per core
KC = C // 128              # contraction chunks
PY, TX = 8, 12             # block: PY rows x TX cols = 96 output pixels
BY, BX = H // PY, W // TX  # 12 x 6 blocks
GB = 2                     # by-rows per output group
NG = BY // GB              # 6 groups
ND = (2 * MD + 1) ** 2     # 81 displacements

# Per-image column layout of the clamped windows.
_BLK = {}        # (by, bx) -> (group, off within group, rv, cv, r0, c0)
_G_COLS = []     # columns per group
for _g in range(NG):
    _gc = 0
    for _h in range(GB):
        _by = _g * GB + _h
        for _bx in range(BX):
            _r0 = max(0, _by * PY - MD)
            _r1 = min(H, _by * PY + PY + MD)
            _c0 = max(0, _bx * TX - MD)
            _c1 = min(W, _bx * TX + TX + MD)
            _BLK[_by, _bx] = (_g, _gc, _r1 - _r0, _c1 - _c0, _r0, _c0)
            _gc += (_r1 - _r0) * (_c1 - _c0)
    _G_COLS.append(_gc)
_G_OFF = [sum(_G_COLS[:g]) for g in range(NG)]
TOT_COLS = sum(_G_COLS)    # 25024
GMAX = max(_G_COLS)        # 4352

_cache = {}


def _build(repeat: int = 1):
    f32 = mybir.dt.float32
    bf16 = mybir.dt.bfloat16
    nc = bacc.Bacc(None, target_bir_lowering=False, debug=False)

    in1_d = nc.dram_tensor("input1", [BPC, C, H, W], f32, kind="ExternalInput")
    in2_d = nc.dram_tensor("input2", [BPC, C, H, W], f32, kind="ExternalInput")
    out_d = nc.dram_tensor("out", [BPC, PY * TX, TOT_COLS], bf16, kind="ExternalOutput")

    with tile.TileContext(nc) as tc:
        with (
            tc.tile_pool(name="inputs", bufs=1) as inp,
            tc.tile_pool(name="chunk", bufs=2) as ch_pool,
            tc.tile_pool(name="stage", bufs=3) as st_pool,
            tc.tile_pool(name="psum", bufs=8, space="PSUM") as psum_pool,
        ):
            HROWS = H // 2  # half-image chunk for in1 staging
            in1_blk, img2 = {}, {}
            for b in range(BPC):
                for k in range(KC):
                    in1_blk[b, k] = inp.tile(
                        [128, H * W], bf16, name=f"i1b_{b}_{k}", tag=f"i1b_{b}_{k}"
                    )
                    img2[b, k] = inp.tile(
                        [128, H * W], bf16, name=f"i2_{b}_{k}", tag=f"i2_{b}_{k}"
                    )

            for _rep in range(repeat):
                # large contiguous casting loads (SWDGE), batch-major so
                # batch 0 compute starts while batch 1 still streams in.
                # in1 is staged through half-image chunks and engine-copied
                # to block-major: free index ((by*BX+bx)*PY+yy)*TX+xx.
                # loads are split into row-halves, ordered so each batch's
                # top-half groups become compute-ready while its bottom half
                # still streams in (keeps DMA_ENGINES saturated at the tail).
                # in2 splits at row 52 (group g2's halo needs rows up to 51).
                cpy = 0

                def load_in2(b, k, s0, s1):
                    c0 = k * 128
                    nc.gpsimd.dma_start(
                        img2[b, k][:, s0 * W : s1 * W],
                        in2_d[b, c0:c0 + 128, s0:s1, :],
                    )

                def load_in1(b, k, r0, r1):
                    nonlocal cpy
                    c0 = k * 128
                    ch = ch_pool.tile([128, 32 * W], bf16, tag="ch")
                    nc.gpsimd.dma_start(
                        ch[:, 0 : (r1 - r0) * W],
                        in1_d[b, c0:c0 + 128, r0:r1, :],
                    )
                    chv = ch[:, 0 : (r1 - r0) * W].rearrange(
                        "p (y bx xx) -> p y bx xx", bx=BX, xx=TX
                    )
                    for by in range(r0 // PY, r1 // PY):
                        src = chv[:, (by * PY - r0):(by * PY - r0 + PY)]
                        src = src.rearrange("p y bx xx -> p bx y xx")
                        dst = in1_blk[b, k][
                            :, by * PY * W : (by + 1) * PY * W
                        ].rearrange("p (bx y xx) -> p bx y xx", bx=BX, y=PY)
                        if cpy % 2 == 0:
                            nc.vector.tensor_copy(dst, src)
                        else:
                            nc.scalar.copy(dst, src)
                        cpy += 1

                # loads arrive in thirds (piece p enables groups 2p, 2p+1
                # of a batch: in1 rows < 32p+32, in2 halo rows < 36+32p), so
                # compute starts early and the out-DMA backlog stays ahead
                # of the drain.
                I1P = [(0, 32), (32, 64), (64, 96)]
                I2P = [(0, 36), (36, 68), (68, 96)]
                for p in range(3):
                    for b in range(BPC):
                        for k in range(KC):
                            load_in2(b, k, *I2P[p])
                            load_in1(b, k, *I1P[p])

                cnt = 0
                # group order matches load-piece arrival.
                SCHED = [(b, g) for gr in ((0, 1), (2, 3), (4, 5))
                         for b in range(BPC) for g in gr]
                for (b, g) in SCHED:
                    if True:
                        stg = st_pool.tile([PY * TX, GMAX], bf16, tag="stg")
                        for h in range(GB):
                            by = g * GB + h
                            for bx in range(BX):
                                _, boff, rv, cv, r0, c0 = _BLK[by, bx]
                                n = rv * cv
                                ps = psum_pool.tile([PY * TX, 512], f32, tag="ps")
                                for k in range(KC):
                                    blkoff = (by * BX + bx) * PY * TX
                                    lhsT = in1_blk[b, k][
                                        :, blkoff : blkoff + PY * TX
                                    ]
                                    v2 = img2[b, k][:].rearrange(
                                        "p (y x) -> p y x", y=H
                                    )
                                    rhs = v2[:, r0 : r0 + rv, c0 : c0 + cv]
                                    nc.tensor.matmul(
                                        ps[:, 0:n], lhsT, rhs,
                                        start=(k == 0), stop=(k == KC - 1),
                                    )
                                dst = stg[:, boff : boff + n]
                                if cnt % 2 == 0:
                                    nc.scalar.copy(dst, ps[:, 0:n])
                                else:
                                    nc.vector.tensor_copy(dst, ps[:, 0:n])
                                cnt += 1
                        gcols = _G_COLS[g]
                        nc.sync.dma_start(
                            out_d[b, :, _G_OFF[g] : _G_OFF[g] + gcols],
                            stg[:, 0:gcols],
                        )

    nc.compile()
    return nc


def _gather_tables():
    """Host gather indices: out[b, d, y, x] = dev[b, P[y, x], COL[d, y, x]]
    (masked).  dev is the device's [128, TOT_COLS] window dump per batch."""
    if "tables" in _cache:
        return _cache["tables"]
    yy, xx = np.meshgrid(np.arange(H), np.arange(W), indexing="ij")
    P = (yy % PY) * TX + (xx % TX)  # [96, 96]
    COL = np.zeros((ND, H, W), dtype=np.int64)
    MASK = np.zeros((ND, H, W), dtype=bool)
    goff_arr = np.zeros((H, W), dtype=np.int64)
    boff_arr = np.zeros((H, W), dtype=np.int64)
    cv_arr = np.zeros((H, W), dtype=np.int64)
    r0_arr = np.zeros((H, W), dtype=np.int64)
    c0_arr = np.zeros((H, W), dtype=np.int64)
    for by in range(BY):
        for bx in range(BX):
            g, boff, rv, cv, r0, c0 = _BLK[by, bx]
            sl = (slice(by * PY, (by + 1) * PY), slice(bx * TX, (bx + 1) * TX))
            goff_arr[sl] = _G_OFF[g]
            boff_arr[sl] = boff
            cv_arr[sl] = cv
            r0_arr[sl] = r0
            c0_arr[sl] = c0
    for di in range(-MD, MD + 1):
        for dj in range(-MD, MD + 1):
            d = (di + MD) * (2 * MD + 1) + (dj + MD)
            ry = yy + di
            rx = xx + dj
            ok = (ry >= 0) & (ry < H) & (rx >= 0) & (rx < W)
            col = goff_arr + boff_arr + (ry - r0_arr) * cv_arr + (rx - c0_arr)
            COL[d] = np.where(ok, col, 0)
            MASK[d] = ok
    _cache["tables"] = (P, COL, MASK)
    return _cache["tables"]


def kernel(input1: np.ndarray, input2: np.ndarray) -> np.ndarray:
    input1 = np.ascontiguousarray(input1, dtype=np.float32)
    input2 = np.ascontiguousarray(input2, dtype=np.float32)
    if "nc" not in _cache:
        _cache["nc"] = _build()
    nc = _cache["nc"]

    in_maps = [
        {
            "input1": input1[i * BPC : (i + 1) * BPC],
            "input2": input2[i * BPC : (i + 1) * BPC],
        }
        for i in range(NCORES)
    ]
    res = bass_utils.run_bass_kernel_spmd(nc, in_maps, core_ids=list(range(NCORES)))
    _cache["last_results"] = res

    dev = np.concatenate(
        [np.asarray(r["out"]).astype(np.float32) for r in res.results], axis=0
    )  # [B, 128, TOT_COLS]
    P, COL, MASK = _gather_tables()
    out = dev[:, P[np.newaxis, :, :], COL]  # [B, ND, H, W]
    out = np.where(MASK, out, np.float32(0.0))  # NaN-safe for x-halo garbage
    out *= np.float32(1.0 / C)
    return np.ascontiguousarray(out, dtype=np.float32)
